# revision 28
# baseline (speedup 1.0000x reference)
"""Multi-head attention (B=8, N=1024, D=768, H=12, softmax over full dim-scaled
scores) on 8 Trainium2 NeuronCores, data-parallel over the batch dimension:
core b computes batch element b end-to-end; no collectives.

Per-core pipeline (all matmuls bf16 inputs, fp32 PSUM accumulation):
  1. Host supplies x[b] pre-transposed (feature-major xT [768, 1024]) and the
     weights pre-cast to bf16.
  2. qkT [1536, 1024] = W_qk^T @ x^T   (feature-major q,k)
     v    [1024, 768]  = x @ W_v       stored as padded per-(jt, pair) blocks
     [v_A | 1 | v_B | 1 | zeros] (193 cols) so each head's PV stationary is a
     full 128-column slice (enables the fast weight-load path).
  3. Per head pair: scoresT[j, i] = k_h^T q_h (K=64 row-packed), exp via
     ScalarE with scale=D^-0.5 folded in, PV chains accumulate
     out_augT[d+1, i] = [v_h | 1]^T @ expT; row 64 is the softmax
     denominator. Normalize via reciprocal + partition_broadcast + multiply
     into aoT [768, 1024] (feature-major).
  4. out = aoT^T @ W_out + b_out, two passes: ct 0..3 + bias run as PE filler
     during the last pair's ScalarE-bound exp phase; ct 4..5 drain at the end.
     Output stored bf16 (upcast on host).
"""

import numpy as np
import ml_dtypes

import concourse.bass as bass
import concourse.bacc as bacc
import concourse.tile as tile
from concourse import mybir
from concourse.bass_utils import run_bass_kernel_spmd

f32 = mybir.dt.float32
bf16 = mybir.dt.bfloat16

B = 8
N = 1024
D = 768
H = 12
DH = 64
SCALE = float(D) ** -0.5
NT = N // 128   # 8 sequence tiles
KT = D // 128   # 6 feature tiles
NPAIR = H // 2  # 6 head pairs
VW = 193        # padded v block width: [v_A(64) | 1 | v_B(64) | 1 | zeros(63)]


def build_bass():
    nc = bacc.Bacc("TRN2", target_bir_lowering=False, debug=False, num_devices=B)
    xT_d = nc.dram_tensor("xT", [D, N], bf16, kind="ExternalInput")
    wqkv_d = nc.dram_tensor("wqkv", [D, 3 * D], bf16, kind="ExternalInput")
    wo_d = nc.dram_tensor("wo", [D, D], bf16, kind="ExternalInput")
    bo_d = nc.dram_tensor("bo", [D], f32, kind="ExternalInput")
    out_d = nc.dram_tensor("out", [N, D], bf16, kind="ExternalOutput")

    with tile.TileContext(nc) as tc:
        with tc.tile_pool(name="persist", bufs=1) as pp:
            # persistent SBUF tensors
            wq_sb = pp.tile([128, KT, 3 * D], bf16)    # W_qkv feature tiles
            xT_sb = pp.tile([128, KT, N], bf16)        # x^T feature tiles
            wo_sb = pp.tile([128, KT, D], bf16)        # W_out feature tiles
            qkT = pp.tile([128, 2 * KT, N], bf16)      # q,k feature-major
            vaug = pp.tile([128, NT, NPAIR, VW], bf16)  # padded v blocks
            aoT = pp.tile([128, KT, N], bf16)          # attention out, f-major
            osb_all = pp.tile([128, NT, 2, 384], bf16)  # out-proj pass1 partial
            bias_f32 = pp.tile([1, D], f32)
            bias_bf = pp.tile([1, D], bf16)
            ones_col = pp.tile([1, 128], bf16)

            # ---- input DMAs. Each dma_start costs ~0.6 us of issue time on
            # its issuing engine, so the startup-critical ones are spread
            # across three engines (sync/gpsimd/scalar) to overlap issue.
            for kt in range(KT):
                nc.sync.dma_start(
                    out=xT_sb[:, kt, :], in_=xT_d[kt * 128:(kt + 1) * 128, :]
                )
            for kt in range(KT):  # pair-0 q columns
                nc.gpsimd.dma_start(
                    out=wq_sb[:, kt, 0:128],
                    in_=wqkv_d[kt * 128:(kt + 1) * 128, 0:128],
                )
            for kt in range(KT):  # pair-0 k columns
                nc.scalar.dma_start(
                    out=wq_sb[:, kt, KT * 128:(KT + 1) * 128],
                    in_=wqkv_d[kt * 128:(kt + 1) * 128, KT * 128:(KT + 1) * 128],
                )
            for kt in range(KT):  # q rest (gpsimd issue is cheap, ~25ns)
                nc.gpsimd.dma_start(
                    out=wq_sb[:, kt, 128:D],
                    in_=wqkv_d[kt * 128:(kt + 1) * 128, 128:D],
                )
            for kt in range(KT):  # v columns (pair-0 fillers)
                nc.scalar.dma_start(
                    out=wq_sb[:, kt, 2 * D:], in_=wqkv_d[kt * 128:(kt + 1) * 128, 2 * D:]
                )
            for kt in range(KT):  # k rest
                nc.gpsimd.dma_start(
                    out=wq_sb[:, kt, D + 128:2 * D],
                    in_=wqkv_d[kt * 128:(kt + 1) * 128, D + 128:2 * D],
                )
            for kt in range(KT):  # needed only from the last pair onward
                nc.scalar.dma_start(
                    out=wo_sb[:, kt, :], in_=wo_d[kt * 128:(kt + 1) * 128, :]
                )
            bo_ap = bo_d[:]
            nc.scalar.dma_start(
                out=bias_f32,
                in_=bass.AP(tensor=bo_ap.tensor, offset=bo_ap.offset,
                            ap=[[0, 1]] + list(bo_ap.ap)),
            )
            nc.vector.tensor_copy(out=bias_bf, in_=bias_f32)
            nc.gpsimd.memset(ones_col, 1.0)
            # vaug padding: ones at cols 64 and 129, zeros at 130:193
            nc.vector.memset(vaug[:, :, :, DH], 1.0)
            nc.vector.memset(vaug[:, :, :, 2 * DH + 1], 1.0)
            nc.vector.memset(vaug[:, :, :, 2 * DH + 2:], 0.0)
            # dummy activation: pulls the exp ACT-table load (~2.7 us) into
            # the initial DMA wait instead of the first real exp
            warm = pp.tile([1, 2], f32)
            nc.vector.memset(warm, 0.0)
            nc.scalar.activation(out=warm, in_=warm,
                                 func=mybir.ActivationFunctionType.Exp)

            # ---- stages B+C: qkv projections interleaved with attention.
            with tc.tile_pool(name="sbC", bufs=2) as sbC, \
                 tc.tile_pool(name="sbAug", bufs=2) as sbAug, \
                 tc.tile_pool(name="sbN", bufs=1) as sbN, \
                 tc.tile_pool(name="sbDo", bufs=3) as sbDo, \
                 tc.tile_pool(name="psP", bufs=2, space="PSUM") as psP, \
                 tc.tile_pool(name="psS", bufs=1, space="PSUM") as psS, \
                 tc.tile_pool(name="psV", bufs=1, space="PSUM") as psV:

                def emit_qk_item(m):
                    # qkT[:, m, :] = W_qk[:, m-cols]^T @ x^T
                    for it in range(2):
                        ps = psP.tile([128, 512], f32, tag="proj", name="ps")
                        for kt in range(KT):
                            nc.tensor.matmul(
                                ps,
                                wq_sb[:, kt, m * 128:(m + 1) * 128],
                                xT_sb[:, kt, it * 512:(it + 1) * 512],
                                start=(kt == 0), stop=(kt == KT - 1),
                            )
                        nc.vector.tensor_copy(
                            out=qkT[:, m, it * 512:(it + 1) * 512], in_=ps
                        )

                def emit_v_item(jt, et):
                    # v[jt-tile, heads 6et..6et+5] = x @ W_v into padded blocks
                    ps = psP.tile([128, 384], f32, tag="proj", name="ps")
                    for kt in range(KT):
                        nc.tensor.matmul(
                            ps,
                            xT_sb[:, kt, jt * 128:(jt + 1) * 128],
                            wq_sb[:, kt, 2 * D + et * 384: 2 * D + (et + 1) * 384],
                            start=(kt == 0), stop=(kt == KT - 1),
                        )
                    # heads 6et..6et+5 -> (pair r, parity q) blocks
                    dst = vaug[:, jt, 3 * et:3 * et + 3, 0:2 * (DH + 1)]
                    dst = dst.rearrange("p r (q d) -> p r q d", q=2)
                    nc.vector.tensor_copy(
                        out=dst[:, :, :, 0:DH],
                        in_=ps.rearrange("p (r q d) -> p r q d", r=3, q=2),
                    )

                def emit_pass1_item(nt):
                    # out-proj pass 1: bias + ct 0..3 for both 384-halves
                    for et in range(2):
                        ps = psP.tile([128, 384], f32, tag="proj", name="ps")
                        nc.tensor.matmul(
                            ps, ones_col,
                            bias_bf[:, et * 384:(et + 1) * 384],
                            start=True, stop=False,
                        )
                        for ct in range(4):
                            nc.tensor.matmul(
                                ps,
                                aoT[:, ct, nt * 128:(nt + 1) * 128],
                                wo_sb[:, ct, et * 384:(et + 1) * 384],
                                start=False, stop=(ct == 3),
                            )
                        nc.vector.tensor_copy(out=osb_all[:, nt, et, :], in_=ps)

                def emit_pv_group(p, parity, it, st):
                    # one PV accumulation chain for head 2p+parity, i-half it;
                    # the stationary v block is a full 128-col slice (FWL).
                    e = st["e"]
                    ps = st["pv"][parity]
                    off = 1024 * parity + it * 512
                    for jt_ in range(NT):
                        nc.tensor.matmul(
                            ps[:, it * 512:(it + 1) * 512],
                            vaug[:, jt_, p, 65 * parity:65 * parity + 128],
                            e[:, jt_, off:off + 512],
                            start=(jt_ == 0), stop=(jt_ == NT - 1),
                        )

                def emit_pv_evac(p, parity, st, scalar=False):
                    aug = sbAug.tile([DH + 1, 1024], f32, tag="aug")
                    src = st["pv"][parity][0:DH + 1, :]
                    if scalar:
                        nc.scalar.copy(out=aug, in_=src)
                    else:
                        nc.vector.tensor_copy(out=aug, in_=src)
                    st["aug"][parity] = aug

                def emit_norm(p, parity, st):
                    aug = st["aug"][parity]
                    # stage the denominator row at partition 0: the custom-DVE
                    # reciprocal reads the wrong partition on hardware when
                    # given a partition-shifted input (CoreSim accepts it)
                    den = sbN.tile([1, 1024], f32, tag="den")
                    nc.vector.tensor_copy(out=den, in_=aug[DH:DH + 1, :])
                    rcp = sbN.tile([1, 1024], f32, tag="rcp")
                    # denominators are sums of 1024 positive exps (~1e2..1e4):
                    # far from the approx-fast edge cases, and 18 correct bits
                    # is plenty for the softmax normalization
                    nc.vector.reciprocal_approx_fast(out=rcp, in_=den)
                    rbc = sbN.tile([DH, 1024], f32, tag="rbc")
                    nc.gpsimd.partition_broadcast(rbc, rcp)
                    nc.vector.tensor_mul(
                        out=aoT[parity * DH:(parity + 1) * DH, p, :],
                        in0=aug[0:DH, :],
                        in1=rbc,
                    )

                # head-start: q,k of pair 0
                emit_qk_item(0)
                emit_qk_item(KT + 0)

                # Software-pipelined pair loop: iteration p runs the scores +
                # exp of pair p on PE/ACT while interleaving (a) the PV chains
                # of pair p-1 and (b) filler projection work, so the in-order
                # PE queue always has ready work while ScalarE grinds exp.
                prev_st = None
                for p in range(NPAIR):
                    # filler items for this pair's jt loop. sched_off pulls
                    # items earlier in the jt loop (the qk evacuations must
                    # clear DVE before the next pair's scores need them).
                    sched_off = 0
                    if p == 0:
                        filler = [("v", 0, 0), ("v", 1, 0),
                                  ("qk", 1, 0), ("qk", KT + 1, 0)]
                        filler += [("v", jt, 0) for jt in range(2, NT)]
                        filler += [("v", jt, 1) for jt in range(NT)]
                    elif p < NPAIR - 1:
                        filler = [("qk", p + 1, 0), ("qk", KT + p + 1, 0)]
                        sched_off = 2
                    else:
                        # last pair: bulk of the output projection (pass 1
                        # needs only ct 0..3 = pairs 0..3, all normalized)
                        filler = [("p1", nt, 0) for nt in range(NT - 2)]

                    cur_st = {
                        "e": sbC.tile([128, NT, 2048], bf16, tag="expT", name="e"),
                        "pv": None,
                        "aug": {},
                    }
                    if prev_st is not None:
                        # one physical 2-bank slot: parity1 reuses parity0's
                        # banks after its evacuation (WAR dep auto-inserted)
                        prev_st["pv"] = [
                            psV.tile([128, 1024], f32, tag="pv", name="pv0",
                                     bufs=1),
                            psV.tile([128, 1024], f32, tag="pv", name="pv1",
                                     bufs=1),
                        ]

                    def emit_filler(k):
                        kind, a1, a2 = filler[k]
                        if kind == "v":
                            emit_v_item(a1, a2)
                        elif kind == "qk":
                            emit_qk_item(a1)
                        else:
                            emit_pass1_item(a1)

                    fi = 0
                    pv_slots = {1: (0, 0), 3: (0, 1), 5: (1, 0), 7: (1, 1)}
                    for jt in range(NT):
                        # 1) interleaved PV chain of the previous pair
                        if prev_st is not None and jt in pv_slots:
                            parity, it = pv_slots[jt]
                            emit_pv_group(p - 1, parity, it, prev_st)
                            if it == 1:
                                emit_pv_evac(p - 1, parity, prev_st)
                        # 2) filler items: one ahead of the scores
                        n_take = min(
                            ((jt + 1 + sched_off) * len(filler)) // NT,
                            len(filler),
                        ) - fi
                        if prev_st is None and n_take > 0:
                            emit_filler(fi)
                            fi += 1
                            n_take -= 1
                        # 3) scores + exp of the current pair
                        sA = psS.tile([128, N], f32, tag="sA")
                        sB = psS.tile([128, N], f32, tag="sB")
                        for it in range(2):
                            nc.tensor.matmul(
                                sA[:, it * 512:(it + 1) * 512],
                                qkT[0:DH, KT + p, jt * 128:(jt + 1) * 128],
                                qkT[0:DH, p, it * 512:(it + 1) * 512],
                                start=True, stop=True,
                            )
                        for it in range(2):
                            nc.tensor.matmul(
                                sB[:, it * 512:(it + 1) * 512],
                                qkT[DH:128, KT + p, jt * 128:(jt + 1) * 128],
                                qkT[DH:128, p, it * 512:(it + 1) * 512],
                                start=True, stop=True,
                            )
                        nc.scalar.activation(
                            out=cur_st["e"][:, jt, 0:1024], in_=sA,
                            func=mybir.ActivationFunctionType.Exp, scale=SCALE,
                        )
                        nc.scalar.activation(
                            out=cur_st["e"][:, jt, 1024:2048], in_=sB,
                            func=mybir.ActivationFunctionType.Exp, scale=SCALE,
                        )
                        # 4) remaining filler for this jt
                        for _ in range(n_take):
                            emit_filler(fi)
                            fi += 1
                    if prev_st is not None:
                        emit_norm(p - 1, 0, prev_st)
                        emit_norm(p - 1, 1, prev_st)
                    prev_st = cur_st

                # ---- drain: last pair's PV + normalization, overlapped with
                # the held-back pass-1 items, then the ct4/ct5 finish + store.
                p = NPAIR - 1
                prev_st["pv"] = [
                    psV.tile([128, 1024], f32, tag="pv", name="pv0", bufs=1),
                    psV.tile([128, 1024], f32, tag="pv", name="pv1", bufs=1),
                ]
                emit_pv_group(p, 0, 0, prev_st)
                emit_pv_group(p, 0, 1, prev_st)
                emit_pv_evac(p, 0, prev_st, scalar=True)
                emit_norm(p, 0, prev_st)
                emit_pass1_item(NT - 2)
                emit_pv_group(p, 1, 0, prev_st)
                emit_pv_group(p, 1, 1, prev_st)
                emit_pv_evac(p, 1, prev_st, scalar=True)
                emit_norm(p, 1, prev_st)
                emit_pass1_item(NT - 1)

                # pass 2: ct4 + ct5 products, add pass-1 partials, store bf16
                for nt in range(NT):
                    osb = sbDo.tile([128, D], bf16, tag="osb")
                    for et in range(2):
                        ps = psP.tile([128, 384], f32, tag="proj", name="ps")
                        for ct in (4, 5):
                            nc.tensor.matmul(
                                ps,
                                aoT[:, ct, nt * 128:(nt + 1) * 128],
                                wo_sb[:, ct, et * 384:(et + 1) * 384],
                                start=(ct == 4), stop=(ct == 5),
                            )
                        nc.vector.tensor_add(
                            out=osb[:, et * 384:(et + 1) * 384],
                            in0=ps,
                            in1=osb_all[:, nt, et, :],
                        )
                        (nc.sync if et == 0 else nc.scalar).dma_start(
                            out=out_d[nt * 128:(nt + 1) * 128,
                                      et * 384:(et + 1) * 384],
                            in_=osb[:, et * 384:(et + 1) * 384],
                        )
    nc.compile()
    return nc


_CACHE = {}


def _get_nc():
    if "nc" not in _CACHE:
        _CACHE["nc"] = build_bass()
    return _CACHE["nc"]


def _make_in_maps(x, w_qkv, w_out, b_out):
    bf = ml_dtypes.bfloat16
    x = np.asarray(x, dtype=np.float32)
    wq_bf = np.ascontiguousarray(np.asarray(w_qkv, dtype=np.float32)).astype(bf)
    wo_bf = np.ascontiguousarray(np.asarray(w_out, dtype=np.float32)).astype(bf)
    bo = np.ascontiguousarray(np.asarray(b_out, dtype=np.float32))
    in_maps = []
    for b in range(B):
        xT = np.ascontiguousarray(x[b].T).astype(bf)
        in_maps.append({"xT": xT, "wqkv": wq_bf, "wo": wo_bf, "bo": bo})
    return in_maps


def kernel(x, w_qkv, w_out, b_out):
    nc = _get_nc()
    in_maps = _make_in_maps(x, w_qkv, w_out, b_out)
    res = run_bass_kernel_spmd(nc, in_maps, list(range(B)))
    return np.stack([res.results[b]["out"] for b in range(B)]).astype(np.float32)


# ---------------------------------------------------------------------------
# profiling helper (used by test.py only; safe no-op fallback if the axon
# NTFF hook infrastructure is unavailable)
def _install_profhook():
    import sys
    import types

    if "antenv.axon_hooks" in sys.modules:
        return True
    try:
        import antenv
        from trn_agent_boot.trn_boot import _ntff_profile_via_ctypes

        hook = _ntff_profile_via_ctypes("/opt/axon/libaxon_pjrt.so")
        mod = types.ModuleType("antenv.axon_hooks")
        mod._hook = hook
        mod.get_axon_ntff_profile_hook = lambda: mod._hook

        def _set(h):
            mod._hook = h

        mod.set_axon_ntff_profile_hook = _set
        sys.modules["antenv.axon_hooks"] = mod
        antenv.axon_hooks = mod

        import concourse.bass_utils as bu

        bu.upload_artifacts = lambda tmpdir: f"local:{tmpdir}"
        return True
    except Exception as e:  # pragma: no cover
        print(f"profhook install failed: {e}")
        return False


def run_traced(x, w_qkv, w_out, b_out, tmpdir=None):
    """Run with NTFF profiling; returns (out, exec_time_ns, results_obj)."""
    traced = _install_profhook()
    nc = _get_nc()
    in_maps = _make_in_maps(x, w_qkv, w_out, b_out)
    res = run_bass_kernel_spmd(
        nc, in_maps, list(range(B)), trace=traced, tmpdir=tmpdir
    )
    out = np.stack([res.results[b]["out"] for b in range(B)]).astype(np.float32)
    return out, res.exec_time_ns, res


# revision 30
# speedup vs baseline: 1.1690x; 1.1690x over previous
"""Multi-head attention (B=8, N=1024, D=768, H=12, softmax over full dim-scaled
scores) on 8 Trainium2 NeuronCores, data-parallel over the batch dimension:
core b computes batch element b end-to-end; no collectives.

Per-core pipeline (all matmuls bf16 inputs, fp32 PSUM accumulation):
  1. Host supplies x[b] pre-transposed (feature-major xT [768, 1024]) and the
     weights pre-cast to bf16.
  2. qkT [1536, 1024] = W_qk^T @ x^T   (feature-major q,k)
     v    [1024, 768]  = x @ W_v       stored as padded per-(jt, pair) blocks
     [v_A | 1 | v_B | 1 | zeros] (193 cols) so each head's PV stationary is a
     full 128-column slice (enables the fast weight-load path).
  3. Per head pair: scoresT[j, i] = k_h^T q_h (K=64 row-packed), exp via
     ScalarE with scale=D^-0.5 folded in, PV chains accumulate
     out_augT[d+1, i] = [v_h | 1]^T @ expT; row 64 is the softmax
     denominator. Normalize via staged reciprocal_approx_fast +
     partition_broadcast + multiply into aoT [768, 1024] (feature-major).
  4. out = aoT^T @ W_out + b_out, two passes: ct 0..3 + bias run as PE filler
     during the last pair's ScalarE-bound exp phase; ct 4..5 drain at the end.
     Output stored bf16 (upcast on host).

Hardware constraints learned the hard way (do not regress):
  - GPSIMD (Pool) has no PSUM port: never read psum tiles from nc.gpsimd.
  - The custom-DVE reciprocal ops read the wrong partition on hardware when
    the input AP has a nonzero base partition (CoreSim accepts it): stage
    the denominator row to partition 0 first.
  - Matmul accumulation chains must stay contiguous per psum bank;
    interleaving two chains matmul-by-matmul regresses PE cadence.
"""

import numpy as np
import ml_dtypes

import concourse.bass as bass
import concourse.bacc as bacc
import concourse.tile as tile
from concourse import mybir
from concourse.bass_utils import run_bass_kernel_spmd

f32 = mybir.dt.float32
bf16 = mybir.dt.bfloat16

B = 8
N = 1024
D = 768
H = 12
DH = 64
SCALE = float(D) ** -0.5
NT = N // 128   # 8 sequence tiles
KT = D // 128   # 6 feature tiles
NPAIR = H // 2  # 6 head pairs
VW = 193        # padded v block width: [v_A(64) | 1 | v_B(64) | 1 | zeros(63)]


def build_bass():
    nc = bacc.Bacc("TRN2", target_bir_lowering=False, debug=False, num_devices=B)
    xT_d = nc.dram_tensor("xT", [D, N], bf16, kind="ExternalInput")
    wqkv_d = nc.dram_tensor("wqkv", [D, 3 * D], bf16, kind="ExternalInput")
    wo_d = nc.dram_tensor("wo", [D, D], bf16, kind="ExternalInput")
    bo_d = nc.dram_tensor("bo", [D], f32, kind="ExternalInput")
    out_d = nc.dram_tensor("out", [N, D], bf16, kind="ExternalOutput")

    with tile.TileContext(nc) as tc:
        with tc.tile_pool(name="persist", bufs=1) as pp:
            # persistent SBUF tensors
            wq_sb = pp.tile([128, KT, 3 * D], bf16)    # W_qkv feature tiles
            xT_sb = pp.tile([128, KT, N], bf16)        # x^T feature tiles
            wo_sb = pp.tile([128, KT, D], bf16)        # W_out feature tiles
            qkT = pp.tile([128, 2 * KT, N], bf16)      # q,k feature-major
            vaug = pp.tile([128, NT, NPAIR, VW], bf16)  # padded v blocks
            aoT = pp.tile([128, KT, N], bf16)          # attention out, f-major
            osb_all = pp.tile([128, NT, 2, 384], bf16)  # out-proj pass1 partial
            bias_f32 = pp.tile([1, D], f32)
            bias_bf = pp.tile([1, D], bf16)
            ones_col = pp.tile([1, 128], bf16)

            # ---- input DMAs. Each dma_start costs ~0.6 us of issue time on
            # its issuing engine, so the startup-critical ones are spread
            # across three engines (sync/gpsimd/scalar) to overlap issue.
            for kt in range(KT):
                nc.sync.dma_start(
                    out=xT_sb[:, kt, :], in_=xT_d[kt * 128:(kt + 1) * 128, :]
                )
            for kt in range(KT):  # pair-0 q columns
                nc.gpsimd.dma_start(
                    out=wq_sb[:, kt, 0:128],
                    in_=wqkv_d[kt * 128:(kt + 1) * 128, 0:128],
                )
            for kt in range(KT):  # pair-0 k columns
                nc.scalar.dma_start(
                    out=wq_sb[:, kt, KT * 128:(KT + 1) * 128],
                    in_=wqkv_d[kt * 128:(kt + 1) * 128, KT * 128:(KT + 1) * 128],
                )
            for kt in range(KT):  # q rest (gpsimd issue is cheap, ~25ns)
                nc.gpsimd.dma_start(
                    out=wq_sb[:, kt, 128:D],
                    in_=wqkv_d[kt * 128:(kt + 1) * 128, 128:D],
                )
            for kt in range(KT):  # v columns (pair-0 fillers)
                nc.scalar.dma_start(
                    out=wq_sb[:, kt, 2 * D:], in_=wqkv_d[kt * 128:(kt + 1) * 128, 2 * D:]
                )
            for kt in range(KT):  # k rest
                nc.gpsimd.dma_start(
                    out=wq_sb[:, kt, D + 128:2 * D],
                    in_=wqkv_d[kt * 128:(kt + 1) * 128, D + 128:2 * D],
                )
            for kt in range(KT):  # needed only from the last pair onward;
                # issued on sync (idle after xT) so ScalarE's sequencer is
                # free for the first exps
                nc.sync.dma_start(
                    out=wo_sb[:, kt, :], in_=wo_d[kt * 128:(kt + 1) * 128, :]
                )
            bo_ap = bo_d[:]
            nc.sync.dma_start(
                out=bias_f32,
                in_=bass.AP(tensor=bo_ap.tensor, offset=bo_ap.offset,
                            ap=[[0, 1]] + list(bo_ap.ap)),
            )
            nc.vector.tensor_copy(out=bias_bf, in_=bias_f32)
            nc.gpsimd.memset(ones_col, 1.0)
            # vaug padding: ones at cols 64 and 129, zeros at 130:193
            nc.vector.memset(vaug[:, :, :, DH], 1.0)
            nc.vector.memset(vaug[:, :, :, 2 * DH + 1], 1.0)
            nc.vector.memset(vaug[:, :, :, 2 * DH + 2:], 0.0)
            # dummy activation: pulls the exp ACT-table load (~2.7 us) into
            # the initial DMA wait instead of the first real exp
            warm = pp.tile([1, 2], f32)
            nc.vector.memset(warm, 0.0)
            nc.scalar.activation(out=warm, in_=warm,
                                 func=mybir.ActivationFunctionType.Exp)

            # ---- stages B+C: qkv projections interleaved with attention.
            with tc.tile_pool(name="sbC", bufs=2) as sbC, \
                 tc.tile_pool(name="sbAug", bufs=2) as sbAug, \
                 tc.tile_pool(name="sbN", bufs=1) as sbN, \
                 tc.tile_pool(name="sbDo", bufs=3) as sbDo, \
                 tc.tile_pool(name="psP", bufs=2, space="PSUM") as psP, \
                 tc.tile_pool(name="psS", bufs=1, space="PSUM") as psS, \
                 tc.tile_pool(name="psV", bufs=1, space="PSUM") as psV:

                def emit_qk_item(m):
                    # qkT[:, m, :] = W_qk[:, m-cols]^T @ x^T
                    for it in range(2):
                        ps = psP.tile([128, 512], f32, tag="proj", name="ps")
                        for kt in range(KT):
                            nc.tensor.matmul(
                                ps,
                                wq_sb[:, kt, m * 128:(m + 1) * 128],
                                xT_sb[:, kt, it * 512:(it + 1) * 512],
                                start=(kt == 0), stop=(kt == KT - 1),
                            )
                        nc.vector.tensor_copy(
                            out=qkT[:, m, it * 512:(it + 1) * 512], in_=ps
                        )

                def emit_v_item(jt, et):
                    # v[jt-tile, heads 6et..6et+5] = x @ W_v into padded blocks
                    ps = psP.tile([128, 384], f32, tag="proj", name="ps")
                    for kt in range(KT):
                        nc.tensor.matmul(
                            ps,
                            xT_sb[:, kt, jt * 128:(jt + 1) * 128],
                            wq_sb[:, kt, 2 * D + et * 384: 2 * D + (et + 1) * 384],
                            start=(kt == 0), stop=(kt == KT - 1),
                        )
                    # heads 6et..6et+5 -> (pair r, parity q) blocks
                    dst = vaug[:, jt, 3 * et:3 * et + 3, 0:2 * (DH + 1)]
                    dst = dst.rearrange("p r (q d) -> p r q d", q=2)
                    nc.vector.tensor_copy(
                        out=dst[:, :, :, 0:DH],
                        in_=ps.rearrange("p (r q d) -> p r q d", r=3, q=2),
                    )

                def emit_pass1_item(nt):
                    # out-proj pass 1: bias + ct 0..3 for both 384-halves
                    for et in range(2):
                        ps = psP.tile([128, 384], f32, tag="proj", name="ps")
                        nc.tensor.matmul(
                            ps, ones_col,
                            bias_bf[:, et * 384:(et + 1) * 384],
                            start=True, stop=False,
                        )
                        for ct in range(4):
                            nc.tensor.matmul(
                                ps,
                                aoT[:, ct, nt * 128:(nt + 1) * 128],
                                wo_sb[:, ct, et * 384:(et + 1) * 384],
                                start=False, stop=(ct == 3),
                            )
                        nc.vector.tensor_copy(out=osb_all[:, nt, et, :], in_=ps)

                def emit_pv_group(p, parity, it, st):
                    # one PV accumulation chain for head 2p+parity, i-half it;
                    # the stationary v block is a full 128-col slice (FWL).
                    e = st["e"]
                    ps = st["pv"][parity]
                    off = 1024 * parity + it * 512
                    for jt_ in range(NT):
                        nc.tensor.matmul(
                            ps[:, it * 512:(it + 1) * 512],
                            vaug[:, jt_, p, 65 * parity:65 * parity + 128],
                            e[:, jt_, off:off + 512],
                            start=(jt_ == 0), stop=(jt_ == NT - 1),
                        )

                def emit_pv_evac(p, parity, st, scalar=False):
                    aug = sbAug.tile([DH + 1, 1024], f32, tag="aug")
                    src = st["pv"][parity][0:DH + 1, :]
                    if scalar:
                        nc.scalar.copy(out=aug, in_=src)
                    else:
                        nc.vector.tensor_copy(out=aug, in_=src)
                    st["aug"][parity] = aug

                def emit_norm(p, parity, st):
                    aug = st["aug"][parity]
                    # stage the denominator row at partition 0: the custom-DVE
                    # reciprocal reads the wrong partition on hardware when
                    # given a partition-shifted input (CoreSim accepts it)
                    den = sbN.tile([1, 1024], f32, tag="den")
                    nc.vector.tensor_copy(out=den, in_=aug[DH:DH + 1, :])
                    rcp = sbN.tile([1, 1024], f32, tag="rcp")
                    # denominators are sums of 1024 positive exps (~1e2..1e4):
                    # far from the approx-fast edge cases, and 18 correct bits
                    # is plenty for the softmax normalization
                    nc.vector.reciprocal_approx_fast(out=rcp, in_=den)
                    rbc = sbN.tile([DH, 1024], f32, tag="rbc")
                    nc.gpsimd.partition_broadcast(rbc, rcp)
                    nc.vector.tensor_mul(
                        out=aoT[parity * DH:(parity + 1) * DH, p, :],
                        in0=aug[0:DH, :],
                        in1=rbc,
                    )

                # head-start: q,k of pair 0
                emit_qk_item(0)
                emit_qk_item(KT + 0)

                # Software-pipelined pair loop: iteration p runs the scores +
                # exp of pair p on PE/ACT while interleaving (a) the PV chains
                # of pair p-1 and (b) filler projection work, so the in-order
                # PE queue always has ready work while ScalarE grinds exp.
                prev_st = None
                for p in range(NPAIR):
                    # filler items for this pair's jt loop. sched_off pulls
                    # items earlier in the jt loop (the qk evacuations must
                    # clear DVE before the next pair's scores need them).
                    sched_off = 0
                    if p == 0:
                        filler = [("v", 0, 0), ("v", 1, 0),
                                  ("qk", 1, 0), ("qk", KT + 1, 0)]
                        filler += [("v", jt, 0) for jt in range(2, NT)]
                        filler += [("v", jt, 1) for jt in range(NT)]
                    elif p < NPAIR - 1:
                        filler = [("qk", p + 1, 0), ("qk", KT + p + 1, 0)]
                        sched_off = 2
                    else:
                        # last pair: bulk of the output projection (pass 1
                        # needs only ct 0..3 = pairs 0..3, all normalized)
                        filler = [("p1", nt, 0) for nt in range(NT - 2)]

                    cur_st = {
                        "e": sbC.tile([128, NT, 2048], bf16, tag="expT", name="e"),
                        "pv": None,
                        "aug": {},
                    }
                    if prev_st is not None:
                        # one physical 2-bank slot: parity1 reuses parity0's
                        # banks after its evacuation (WAR dep auto-inserted)
                        prev_st["pv"] = [
                            psV.tile([128, 1024], f32, tag="pv", name="pv0",
                                     bufs=1),
                            psV.tile([128, 1024], f32, tag="pv", name="pv1",
                                     bufs=1),
                        ]

                    def emit_filler(k):
                        kind, a1, a2 = filler[k]
                        if kind == "v":
                            emit_v_item(a1, a2)
                        elif kind == "qk":
                            emit_qk_item(a1)
                        else:
                            emit_pass1_item(a1)

                    fi = 0
                    pv_slots = {1: (0, 0), 3: (0, 1), 5: (1, 0), 7: (1, 1)}
                    for jt in range(NT):
                        # 1) interleaved PV chain of the previous pair
                        if prev_st is not None and jt in pv_slots:
                            parity, it = pv_slots[jt]
                            emit_pv_group(p - 1, parity, it, prev_st)
                            if it == 1:
                                emit_pv_evac(p - 1, parity, prev_st)
                        # 2) filler items: one ahead of the scores
                        n_take = min(
                            ((jt + 1 + sched_off) * len(filler)) // NT,
                            len(filler),
                        ) - fi
                        if prev_st is None and n_take > 0:
                            emit_filler(fi)
                            fi += 1
                            n_take -= 1
                        # 3) scores + exp of the current pair
                        sA = psS.tile([128, N], f32, tag="sA")
                        sB = psS.tile([128, N], f32, tag="sB")
                        for it in range(2):
                            nc.tensor.matmul(
                                sA[:, it * 512:(it + 1) * 512],
                                qkT[0:DH, KT + p, jt * 128:(jt + 1) * 128],
                                qkT[0:DH, p, it * 512:(it + 1) * 512],
                                start=True, stop=True,
                            )
                        for it in range(2):
                            nc.tensor.matmul(
                                sB[:, it * 512:(it + 1) * 512],
                                qkT[DH:128, KT + p, jt * 128:(jt + 1) * 128],
                                qkT[DH:128, p, it * 512:(it + 1) * 512],
                                start=True, stop=True,
                            )
                        nc.scalar.activation(
                            out=cur_st["e"][:, jt, 0:1024], in_=sA,
                            func=mybir.ActivationFunctionType.Exp, scale=SCALE,
                        )
                        nc.scalar.activation(
                            out=cur_st["e"][:, jt, 1024:2048], in_=sB,
                            func=mybir.ActivationFunctionType.Exp, scale=SCALE,
                        )
                        # 4) remaining filler for this jt
                        for _ in range(n_take):
                            emit_filler(fi)
                            fi += 1
                    if prev_st is not None:
                        emit_norm(p - 1, 0, prev_st)
                        emit_norm(p - 1, 1, prev_st)
                    prev_st = cur_st

                # ---- drain: last pair's PV + normalization, overlapped with
                # the held-back pass-1 items, then the ct4/ct5 finish + store.
                p = NPAIR - 1
                prev_st["pv"] = [
                    psV.tile([128, 1024], f32, tag="pv", name="pv0", bufs=1),
                    psV.tile([128, 1024], f32, tag="pv", name="pv1", bufs=1),
                ]
                emit_pv_group(p, 0, 0, prev_st)
                emit_pv_group(p, 0, 1, prev_st)
                emit_pv_evac(p, 0, prev_st, scalar=True)
                emit_norm(p, 0, prev_st)
                emit_pass1_item(NT - 2)
                emit_pv_group(p, 1, 0, prev_st)
                emit_pv_group(p, 1, 1, prev_st)
                emit_pv_evac(p, 1, prev_st, scalar=True)
                emit_norm(p, 1, prev_st)
                emit_pass1_item(NT - 1)

                # pass 2: ct4 + ct5 products, add pass-1 partials, store bf16
                for nt in range(NT):
                    osb = sbDo.tile([128, D], bf16, tag="osb")
                    for et in range(2):
                        ps = psP.tile([128, 384], f32, tag="proj", name="ps")
                        for ct in (4, 5):
                            nc.tensor.matmul(
                                ps,
                                aoT[:, ct, nt * 128:(nt + 1) * 128],
                                wo_sb[:, ct, et * 384:(et + 1) * 384],
                                start=(ct == 4), stop=(ct == 5),
                            )
                        nc.vector.tensor_add(
                            out=osb[:, et * 384:(et + 1) * 384],
                            in0=ps,
                            in1=osb_all[:, nt, et, :],
                        )
                        (nc.sync if et == 0 else nc.scalar).dma_start(
                            out=out_d[nt * 128:(nt + 1) * 128,
                                      et * 384:(et + 1) * 384],
                            in_=osb[:, et * 384:(et + 1) * 384],
                        )
    nc.compile()
    return nc


_CACHE = {}


def _get_nc():
    if "nc" not in _CACHE:
        _CACHE["nc"] = build_bass()
    return _CACHE["nc"]


def _make_in_maps(x, w_qkv, w_out, b_out):
    bf = ml_dtypes.bfloat16
    x = np.asarray(x, dtype=np.float32)
    wq_bf = np.ascontiguousarray(np.asarray(w_qkv, dtype=np.float32)).astype(bf)
    wo_bf = np.ascontiguousarray(np.asarray(w_out, dtype=np.float32)).astype(bf)
    bo = np.ascontiguousarray(np.asarray(b_out, dtype=np.float32))
    in_maps = []
    for b in range(B):
        xT = np.ascontiguousarray(x[b].T).astype(bf)
        in_maps.append({"xT": xT, "wqkv": wq_bf, "wo": wo_bf, "bo": bo})
    return in_maps


def kernel(x, w_qkv, w_out, b_out):
    nc = _get_nc()
    in_maps = _make_in_maps(x, w_qkv, w_out, b_out)
    res = run_bass_kernel_spmd(nc, in_maps, list(range(B)))
    return np.stack([res.results[b]["out"] for b in range(B)]).astype(np.float32)


# ---------------------------------------------------------------------------
# profiling helper (used by test.py only; safe no-op fallback if the axon
# NTFF hook infrastructure is unavailable)
def _install_profhook():
    import sys
    import types

    if "antenv.axon_hooks" in sys.modules:
        return True
    try:
        import antenv
        from trn_agent_boot.trn_boot import _ntff_profile_via_ctypes

        hook = _ntff_profile_via_ctypes("/opt/axon/libaxon_pjrt.so")
        mod = types.ModuleType("antenv.axon_hooks")
        mod._hook = hook
        mod.get_axon_ntff_profile_hook = lambda: mod._hook

        def _set(h):
            mod._hook = h

        mod.set_axon_ntff_profile_hook = _set
        sys.modules["antenv.axon_hooks"] = mod
        antenv.axon_hooks = mod

        import concourse.bass_utils as bu

        bu.upload_artifacts = lambda tmpdir: f"local:{tmpdir}"
        return True
    except Exception as e:  # pragma: no cover
        print(f"profhook install failed: {e}")
        return False


def run_traced(x, w_qkv, w_out, b_out, tmpdir=None):
    """Run with NTFF profiling; returns (out, exec_time_ns, results_obj)."""
    traced = _install_profhook()
    nc = _get_nc()
    in_maps = _make_in_maps(x, w_qkv, w_out, b_out)
    res = run_bass_kernel_spmd(
        nc, in_maps, list(range(B)), trace=traced, tmpdir=tmpdir
    )
    out = np.stack([res.results[b]["out"] for b in range(B)]).astype(np.float32)
    return out, res.exec_time_ns, res
